# revision 7
# baseline (speedup 1.0000x reference)
"""Trainium2 Bass kernel for nn_MethodCFGEncoder (gnn_message_passing).

Full computation (see reference):
  1. gather expr-token rows + symbol rows for N_APP occurrence triples
  2. gated update (2 matmul gates), scatter back (last-write-wins)
  3. mean over tokens per expression
  4. gated mix with prev node encodings (+ mask select)

Strategy (data-parallel over expressions, 8 cores, no collectives):
  - Host: dedup occurrences per flat slot (last-write-wins => at most one
    update per slot), bucket by expression shard, compact per-core symbol
    tables (so gather indices fit int16 for dma_gather).
  - Device (per core, dense over all 32768 slots of its shard, transposed
    [feature, slot] layout so the feature contraction sits on partitions):
      f_logit  = Wg1a.T@oldT + Wg1b.T@symT  (+64 on slots with no update)
      cand     = relu(Wc1a.T@oldT + Wc1b.T@symT + bc1)
      new      = cand + sigmoid(f_logit + bg1)*(old - cand)   [f=1 => new=old]
      combined = sum_tokens(new)            (1/32 folded into stage-2 weights)
      out      = cand2 + sigmoid(...)*(prev - cand2)          (stage-2 gate)
  - Matmul operands are float32r (full-rate PE; ~1e-4 rounding) while the
    gate blends read the exact fp32 bits, so only logits see the rounding.
  - Symbol rows arrive via dma_gather (row-major) and are transposed on PE.
"""

import math

import numpy as np

import concourse.bass as bass
import concourse.mybir as mybir
import concourse.tile as tile
from concourse import bacc, bass_utils
from concourse.masks import make_identity

F32 = mybir.dt.float32
F32R = mybir.dt.float32r
I16 = mybir.dt.int16

N_CORES = 8
N_EXPR = 8192
MAX_TOK = 32
D = 256
N_SYM = 50000
N_APP = 262144
BIG = 64.0
GROUP = 512  # slots per inner tile group (fp32 PSUM bank = 512 cols)


def _build_program(epc, table_rows, num_devices, reps=1, ab=()):
    """Build + compile the SPMD bass program.

    epc: expressions per core. table_rows: padded compacted symbol-table rows.
    reps: repeat the whole computation (timing only). ab: ablation flags
    ("gather", "tr", "mmact", "dve") used for perf bisection.
    """
    ab = frozenset(ab)
    spc = epc * MAX_TOK          # slots per core
    ng = spc // GROUP            # inner groups
    ehalves = math.ceil(epc / 512)  # stage-2 column halves

    nc = bacc.Bacc(
        "TRN2", target_bir_lowering=False, debug=False, num_devices=num_devices
    )

    exprT = nc.dram_tensor("exprT", (D, spc), F32R, kind="ExternalInput").ap()
    prevT = nc.dram_tensor("prevT", (D, epc), F32R, kind="ExternalInput").ap()
    symtab = nc.dram_tensor("symtab", (table_rows, D), F32, kind="ExternalInput").ap()
    symidx = nc.dram_tensor("symidx", (128, spc // 16), I16, kind="ExternalInput").ap()
    m1 = nc.dram_tensor("m1", (1, spc), F32R, kind="ExternalInput").ap()
    m2 = nc.dram_tensor("m2", (1, epc), F32R, kind="ExternalInput").ap()
    onesd = nc.dram_tensor("ones", (1, 128), F32R, kind="ExternalInput").ap()
    wd = {
        name: nc.dram_tensor(name, (2 * D, D), F32R, kind="ExternalInput").ap()
        for name in ("wg1", "wc1", "wg2", "wc2")
    }
    bd = {
        name: nc.dram_tensor(name, (D, 1), F32, kind="ExternalInput").ap()
        for name in ("bg1", "bc1", "bg2", "bc2")
    }
    out = nc.dram_tensor("out", (epc, D), F32, kind="ExternalOutput").ap()

    SIG = mybir.ActivationFunctionType.Sigmoid
    REL = mybir.ActivationFunctionType.Relu
    ADD = mybir.AluOpType.add
    AXX = mybir.AxisListType.X

    with tile.TileContext(nc) as tc:
        with (
            tc.tile_pool(name="const", bufs=1) as const,
            tc.tile_pool(name="res", bufs=1) as res,
            tc.tile_pool(name="work", bufs=2) as work,
            tc.tile_pool(name="io", bufs=3) as io,
            tc.tile_pool(name="pg", bufs=1, space="PSUM") as pg,
            tc.tile_pool(name="pt", bufs=2, space="PSUM") as pt,
        ):
            ident = const.tile([128, 128], F32, tag="ident")
            make_identity(nc, ident[:])
            ones1 = const.tile([1, 128], F32R, tag="ones1")
            nc.sync.dma_start(out=ones1[:], in_=onesd[:])

            # stationary weight tiles W[name][k][m]: [K=128 feat, M=128 outfeat]
            W = {}
            for name in ("wg1", "wc1", "wg2", "wc2"):
                W[name] = []
                for k in range(4):
                    row = []
                    for m in range(2):
                        t = const.tile([128, 128], F32R, tag=f"{name}_{k}{m}",
                                       name=f"{name}_{k}{m}")
                        nc.sync.dma_start(
                            out=t[:],
                            in_=wd[name][k * 128:(k + 1) * 128, m * 128:(m + 1) * 128],
                        )
                        row.append(t)
                    W[name].append(row)
            B = {}
            for name in ("bg1", "bc1", "bg2", "bc2"):
                B[name] = []
                for m in range(2):
                    t = const.tile([128, 1], F32, tag=f"{name}_{m}", name=f"{name}_{m}")
                    nc.sync.dma_start(out=t[:], in_=bd[name][m * 128:(m + 1) * 128, :])
                    B[name].append(t)

            idxt = const.tile([128, spc // 16], I16, tag="idxt")
            nc.sync.dma_start(out=idxt[:], in_=symidx[:])
            m2t = const.tile([1, epc], F32R, tag="m2t")
            nc.sync.dma_start(out=m2t[:], in_=m2[:])

            comb = []
            prevS = []
            new2 = []
            for m in range(2):
                comb.append(res.tile([128, epc], F32, tag=f"comb{m}", name=f"comb{m}"))
                p = res.tile([128, epc], F32R, tag=f"prev{m}", name=f"prev{m}")
                nc.sync.dma_start(out=p[:], in_=prevT[m * 128:(m + 1) * 128, :])
                prevS.append(p)
                new2.append(res.tile([128, epc], F32, tag=f"new2_{m}", name=f"new2_{m}"))

            # ablation stand-ins (timing bisection only)
            if "gather" in ab:
                srm_sh = res.tile([128, GROUP // 128, D], F32, name="srm_sh")
                nc.gpsimd.memset(srm_sh[:], 0.25)
            if "tr" in ab:
                sT_sh = []
                for k in range(2):
                    t = res.tile([128, GROUP], F32R, tag=f"sTsh{k}", name=f"sTsh{k}")
                    nc.sync.dma_start(out=t[:], in_=exprT[k * 128:(k + 1) * 128, 0:GROUP])
                    sT_sh.append(t)
            if "mmact" in ab:
                f_sh = res.tile([128, 2 * GROUP], F32, name="f_sh")
                nc.gpsimd.memset(f_sh[:], 0.5)
                c_sh = res.tile([128, 2 * GROUP], F32, name="c_sh")
                nc.gpsimd.memset(c_sh[:], 0.25)
            if "dve" in ab:
                for m in range(2):
                    nc.gpsimd.memset(comb[m][:], 0.125)
                    nc.gpsimd.memset(new2[m][:], 0.125)

            # ---- stage 1: per-pair (2 groups = 1024 slots) gated update ----
            PAIR = 2 * GROUP
            exprTv = exprT.rearrange("(c p) s -> p c s", p=128)
            for _rep in range(reps):
              for gp in range(ng // 2):
                  cp = slice(gp * PAIR, (gp + 1) * PAIR)
                  eTb = io.tile([128, 2, PAIR], F32R, tag="eTb", name="eTb")
                  nc.sync.dma_start(out=eTb[:], in_=exprTv[:, :, cp])
                  m1t = work.tile([1, PAIR], F32R, tag="m1t")
                  nc.sync.dma_start(out=m1t[:], in_=m1[:, cp])

                  if "gather" in ab:
                      srm = srm_sh
                  else:
                      srm = io.tile([128, PAIR // 128, D], F32, tag="srm")
                      nc.gpsimd.dma_gather(
                          srm[:], symtab[:],
                          idxt[:, gp * (PAIR // 16):(gp + 1) * (PAIR // 16)],
                          PAIR, PAIR, D,
                      )

                  f_b = [work.tile([128, PAIR], F32, tag=f"f{m}", name=f"f{m}")
                         for m in range(2)]
                  c_b = [work.tile([128, PAIR], F32, tag=f"c{m}", name=f"c{m}")
                         for m in range(2)]

                  for g2 in range(2):
                      gsl = slice(g2 * GROUP, (g2 + 1) * GROUP)
                      if "tr" in ab:
                          sT = sT_sh
                      else:
                          sT = []
                          for k in range(2):
                              ps = pt.tile([128, GROUP], F32, tag=f"st{k}", name=f"st{k}")
                              for c in range(GROUP // 128):
                                  nc.tensor.transpose(
                                      out=ps[:, c * 128:(c + 1) * 128],
                                      in_=srm[:, g2 * 4 + c, k * 128:(k + 1) * 128],
                                      identity=ident[:],
                                  )
                              t = work.tile([128, GROUP], F32R, tag=f"sT{k}", name=f"sT{k}")
                              nc.scalar.copy(t[:], ps[:])
                              sT.append(t)

                      dataT = [eTb[:, 0, gsl], eTb[:, 1, gsl], sT[0][:], sT[1][:]]
                      for m in range(2):
                          if "mmact" in ab:
                              continue
                          pf = pg.tile([128, GROUP], F32, tag=f"pf{m}", name=f"pf{m}")
                          for k in range(4):
                              nc.tensor.matmul(
                                  out=pf[:], lhsT=W["wg1"][k][m][:], rhs=dataT[k],
                                  start=(k == 0), stop=False,
                              )
                          nc.tensor.matmul(
                              out=pf[:], lhsT=ones1[:], rhs=m1t[:, gsl],
                              start=False, stop=True,
                          )
                          nc.scalar.activation(f_b[m][:, gsl], pf[:], SIG,
                                               bias=B["bg1"][m][:])

                          pc = pg.tile([128, GROUP], F32, tag=f"pc{m}", name=f"pc{m}")
                          for k in range(4):
                              nc.tensor.matmul(
                                  out=pc[:], lhsT=W["wc1"][k][m][:], rhs=dataT[k],
                                  start=(k == 0), stop=(k == 3),
                              )
                          nc.scalar.activation(c_b[m][:, gsl], pc[:], REL,
                                               bias=B["bc1"][m][:])

                  if "mmact" in ab:
                      fs = [f_sh, f_sh]
                      csb = [c_sh, c_sh]
                  else:
                      fs, csb = f_b, c_b
                  if "dve" not in ab:
                      for m in range(2):
                          dd = work.tile([128, PAIR], F32, tag=f"d{m}", name=f"d{m}")
                          nc.gpsimd.tensor_sub(dd[:], eTb[:, m, :].bitcast(F32), csb[m][:])
                          dm = work.tile([128, PAIR], F32, tag=f"dm{m}", name=f"dm{m}")
                          nc.vector.tensor_mul(dm[:], fs[m][:], dd[:])
                          nw = work.tile([128, PAIR], F32, tag=f"nw{m}", name=f"nw{m}")
                          nc.vector.tensor_add(nw[:], csb[m][:], dm[:])
                          nc.vector.tensor_reduce(
                              out=comb[m][:, gp * (PAIR // 32):(gp + 1) * (PAIR // 32)],
                              in_=nw[:].rearrange("p (e t) -> p e t", t=MAX_TOK),
                              axis=AXX, op=ADD,
                          )

              # ---- stage 2: node gate over epc expressions ----
              if "mmact" not in ab:
                  combR = []
                  for m in range(2):
                      cr = res.tile([128, epc], F32R, tag=f"combR{m}", name=f"combR{m}")
                      nc.scalar.copy(cr[:], comb[m][:])
                      combR.append(cr)
              for h in range(ehalves):
                  hs = slice(h * 512, min((h + 1) * 512, epc))
                  hn = hs.stop - hs.start
                  for m in range(2):
                      if "mmact" in ab:
                          f_sb, c_sb = f_sh, c_sh
                      else:
                          data2 = [prevS[0], prevS[1], combR[0], combR[1]]
                          pf = pg.tile([128, GROUP], F32, tag=f"pf{m}", name=f"pf{m}")
                          for k in range(4):
                              nc.tensor.matmul(
                                  out=pf[:, :hn], lhsT=W["wg2"][k][m][:],
                                  rhs=data2[k][:, hs], start=(k == 0), stop=False,
                              )
                          nc.tensor.matmul(
                              out=pf[:, :hn], lhsT=ones1[:], rhs=m2t[:, hs],
                              start=False, stop=True,
                          )
                          f_sb = work.tile([128, GROUP], F32, tag=f"f{m}", name=f"f{m}")
                          nc.scalar.activation(f_sb[:, :hn], pf[:, :hn], SIG,
                                               bias=B["bg2"][m][:])

                          pc = pg.tile([128, GROUP], F32, tag=f"pc{m}", name=f"pc{m}")
                          for k in range(4):
                              nc.tensor.matmul(
                                  out=pc[:, :hn], lhsT=W["wc2"][k][m][:],
                                  rhs=data2[k][:, hs], start=(k == 0), stop=(k == 3),
                              )
                          c_sb = work.tile([128, GROUP], F32, tag=f"c{m}", name=f"c{m}")
                          nc.scalar.activation(c_sb[:, :hn], pc[:, :hn], REL,
                                               bias=B["bc2"][m][:])

                      if "dve" not in ab:
                          dd = work.tile([128, GROUP], F32, tag=f"d{m}", name=f"d{m}")
                          nc.vector.tensor_sub(dd[:, :hn], prevS[m][:, hs].bitcast(F32),
                                               c_sb[:, :hn])
                          dm = work.tile([128, GROUP], F32, tag=f"dm{m}", name=f"dm{m}")
                          nc.vector.tensor_mul(dm[:, :hn], f_sb[:, :hn], dd[:, :hn])
                          nc.vector.tensor_add(new2[m][:, hs], c_sb[:, :hn], dm[:, :hn])

              # ---- transpose back to [expr, D] and store ----
              for c in range(epc // 128):
                  ps = pt.tile([128, GROUP], F32, tag=f"st{c % 2}", name=f"stc{c % 2}")
                  for k in range(2):
                      nc.tensor.transpose(
                          out=ps[:, k * 128:(k + 1) * 128],
                          in_=new2[k][:, c * 128:(c + 1) * 128],
                          identity=ident[:],
                      )
                  osb = work.tile([128, D], F32, tag="osb")
                  nc.scalar.copy(osb[:], ps[:, :D])
                  nc.sync.dma_start(out=out[c * 128:(c + 1) * 128, :], in_=osb[:])

    nc.compile()
    return nc


_PROGRAM_CACHE = {}


def _get_program(epc, table_rows, num_devices):
    key = (epc, table_rows, num_devices)
    if key not in _PROGRAM_CACHE:
        _PROGRAM_CACHE[key] = _build_program(epc, table_rows, num_devices)
    return _PROGRAM_CACHE[key]


def _host_prep(inputs, n_cores=N_CORES):
    """numpy: dedup occurrences, bucket by shard, build per-core input maps."""
    expr = np.ascontiguousarray(np.asarray(inputs["expressions_encodings"], np.float32))
    sym = np.ascontiguousarray(np.asarray(inputs["symbols_encodings"], np.float32))
    prev = np.ascontiguousarray(np.asarray(inputs["prev_cfg_nodes_encodings"], np.float32))
    aei = np.asarray(inputs["app_expr_idx"]).astype(np.int64)
    ati = np.asarray(inputs["app_token_idx"]).astype(np.int64)
    asi = np.asarray(inputs["app_symbol_idx"]).astype(np.int64)
    mask = np.asarray(inputs["cfg_nodes_has_expression_mask"]).astype(np.float32)

    n_expr, max_tok, d = expr.shape
    epc = n_expr // n_cores
    spc = epc * max_tok

    # last-write-wins dedup: highest app index per flat slot
    flat_idx = aei * max_tok + ati
    winner = np.full(n_expr * max_tok, -1, np.int64)
    np.maximum.at(winner, flat_idx, np.arange(flat_idx.shape[0], dtype=np.int64))
    has = winner >= 0
    sym_for_slot = np.where(has, asi[np.clip(winner, 0, None)], 0)

    flat = expr.reshape(-1, d)

    per_core = []
    max_uniq = 0
    for c in range(n_cores):
        sl = slice(c * spc, (c + 1) * spc)
        uniq, inv = np.unique(sym_for_slot[sl], return_inverse=True)
        per_core.append((sl, uniq, inv))
        max_uniq = max(max_uniq, len(uniq))
    table_rows = max(2048, math.ceil(max_uniq / 2048) * 2048)

    w1 = np.asarray(inputs["Wg1"], np.float32)
    c1 = np.asarray(inputs["Wc1"], np.float32)
    w2 = np.asarray(inputs["Wg2"], np.float32).copy()
    c2 = np.asarray(inputs["Wc2"], np.float32).copy()
    w2[d:] *= 1.0 / max_tok  # combined arrives as token SUM, not mean
    c2[d:] *= 1.0 / max_tok
    biases = {
        "bg1": np.asarray(inputs["bg1"], np.float32).reshape(d, 1),
        "bc1": np.asarray(inputs["bc1"], np.float32).reshape(d, 1),
        "bg2": np.asarray(inputs["bg2"], np.float32).reshape(d, 1),
        "bc2": np.asarray(inputs["bc2"], np.float32).reshape(d, 1),
    }

    in_maps = []
    for c in range(n_cores):
        sl, uniq, inv = per_core[c]
        tab = np.zeros((table_rows, d), np.float32)
        tab[: len(uniq)] = sym[uniq]
        idx16 = inv.astype(np.int16).reshape(spc // 16, 16).T  # [16, spc/16]
        idx_rep = np.ascontiguousarray(np.tile(idx16, (8, 1)))  # [128, spc/16]
        m1_c = (BIG * (1.0 - has[sl].astype(np.float32)))[None, :]
        m2_c = (BIG * (1.0 - mask[c * epc:(c + 1) * epc]))[None, :]
        in_maps.append({
            "exprT": np.ascontiguousarray(flat[sl].T),
            "prevT": np.ascontiguousarray(prev[c * epc:(c + 1) * epc].T),
            "symtab": tab,
            "symidx": idx_rep,
            "m1": np.ascontiguousarray(m1_c),
            "m2": np.ascontiguousarray(m2_c),
            "wg1": w1, "wc1": c1, "wg2": w2, "wc2": c2,
            "ones": np.ones((1, 128), np.float32),
            **biases,
        })
    return epc, table_rows, in_maps


def kernel(**inputs):
    epc, table_rows, in_maps = _host_prep(inputs)
    nc = _get_program(epc, table_rows, N_CORES)
    res = bass_utils.run_bass_kernel_spmd(nc, in_maps, core_ids=list(range(N_CORES)))
    return np.concatenate([res.results[c]["out"] for c in range(N_CORES)], axis=0)
